# revision 1
# baseline (speedup 1.0000x reference)
"""Trainium2 Bass kernel for nn_ConditionalRandomField_52913997087452.

Computes sum_b [ gold_path_score(b) - log Z(b) ] for a linear-chain CRF with
B=128, L=1024, T=128, mask all-ones.

Strategy (data-parallel over batch, 16 per core x 8 cores):
  - log-partition Z via the *scaled forward algorithm* in the linear domain:
        pi_0 = exp(logits[0] + start)
        pi_t = (Ehat^T pi_{t-1}) * exp(logits[t])          (end folded into t=L-1)
    with Ehat = exp(transitions - ghat) as a bf16 PE stationary, pi bf16
    tag-major [T, B_core], emissions F = exp(logits) f32 tag-major in SBUF,
    PSUM accumulation in f32. Every RENORM steps a ones-vector matmul on the
    PE computes S[b] = sum_i pi[i,b]; 1/S is folded into a later F column
    (off the critical path) and S is streamed out so the host adds back
    sum_k log S_k. Host finishes: logZ = log(w) + sum log S_k + (L-1)*ghat.
  - The gold-path numerator is a tiny gather-and-sum done on the host.

The kernel builder is cached at module level so repeated kernel() calls
reuse the compiled program.
"""
import sys

if "/opt/trn_rl_repo" not in sys.path:
    sys.path.insert(0, "/opt/trn_rl_repo")

import numpy as np

import concourse.bacc as bacc
import concourse.tile as tile
from concourse import mybir
from concourse.bass_utils import run_bass_kernel_spmd

B = 128
L = 1024
T = 128
NCORES = 8
BPC = B // NCORES       # batch per core
RENORM = 128            # renorm interval (steps)
APPLY_DELAY = 8         # fold 1/S into F column t0 + APPLY_DELAY
NREN = (L - 1) // RENORM


def _build():
    nc = bacc.Bacc("TRN2", target_bir_lowering=False)
    lg = nc.dram_tensor("lg", [BPC * L, T], mybir.dt.float32, kind="ExternalInput")
    eh = nc.dram_tensor("eh", [T, T], mybir.dt.float32, kind="ExternalInput")
    w_out = nc.dram_tensor("w", [1, BPC], mybir.dt.float32, kind="ExternalOutput")
    s_out = nc.dram_tensor("s", [1, NREN * BPC], mybir.dt.float32,
                           kind="ExternalOutput")

    with tile.TileContext(nc) as tc:
        with (
            tc.tile_pool(name="consts", bufs=1) as consts,
            tc.tile_pool(name="fpool", bufs=1) as fpool,
            tc.tile_pool(name="pipool", bufs=4) as pipool,
            tc.tile_pool(name="mmpsum", bufs=4, space="PSUM") as mmpsum,
            tc.tile_pool(name="trpsum", bufs=2, space="PSUM") as trpsum,
            tc.tile_pool(name="spsum", bufs=2, space="PSUM") as spsum,
            tc.tile_pool(name="stage", bufs=4) as stage,
            tc.tile_pool(name="rpool", bufs=2) as rpool,
        ):
            # ---- constants ----
            eh_f32 = consts.tile([T, T], mybir.dt.float32)
            nc.sync.dma_start(out=eh_f32[:], in_=eh[:, :])
            eh_bf = consts.tile([T, T], mybir.dt.bfloat16)
            nc.vector.tensor_copy(out=eh_bf[:], in_=eh_f32[:])
            ones_bf = consts.tile([T, 1], mybir.dt.bfloat16)
            nc.vector.memset(ones_bf[:], 1.0)

            # identity (f32) for PE transpose, built from iotas
            ident = consts.tile([T, T], mybir.dt.float32)
            iota_p = consts.tile([T, 1], mybir.dt.int32)
            nc.gpsimd.iota(iota_p[:], pattern=[[0, 1]], base=0, channel_multiplier=1)
            iota_f = consts.tile([T, T], mybir.dt.int32)
            nc.gpsimd.iota(iota_f[:], pattern=[[1, T]], base=0, channel_multiplier=0)
            iota_pf = consts.tile([T, 1], mybir.dt.float32)
            nc.vector.tensor_copy(out=iota_pf[:], in_=iota_p[:])
            iota_ff = consts.tile([T, T], mybir.dt.float32)
            nc.vector.tensor_copy(out=iota_ff[:], in_=iota_f[:])
            nc.vector.tensor_scalar(
                out=ident[:], in0=iota_ff[:], scalar1=iota_pf[:], scalar2=None,
                op0=mybir.AluOpType.is_equal,
            )

            # ---- F: 8 chunk tiles [T, 128, BPC] f32, filled t-chunk-major so
            # the main loop can start as soon as chunk 0 is ready ----
            NCH = L // 128
            F = []
            for c in range(NCH):
                fc = fpool.tile([T, 128, BPC], mybir.dt.float32, tag=f"F{c}",
                                name=f"F{c}")
                F.append(fc)

            def fcol(t):
                return F[t // 128][:, t % 128, :]

            for tch in range(NCH):
                for b in range(BPC):
                    k = b * NCH + tch  # row-tile index into lg
                    xt = stage.tile([128, T], mybir.dt.float32, name="xt")
                    nc.sync.dma_start(out=xt[:], in_=lg[k * 128:(k + 1) * 128, :])
                    pt = trpsum.tile([T, 128], mybir.dt.float32, name="pt")
                    nc.tensor.transpose(pt[:], xt[:], ident[:])
                    nc.scalar.activation(
                        out=F[tch][:, :, b], in_=pt[:],
                        func=mybir.ActivationFunctionType.Exp,
                    )

            sacc = consts.tile([1, NREN * BPC], mybir.dt.float32)

            # ---- initial pi ----
            pi = pipool.tile([T, BPC], mybir.dt.bfloat16, tag="pi", name="pi")
            nc.vector.tensor_copy(out=pi[:], in_=fcol(0))

            # ---- main recurrence ----
            for t in range(1, L):
                ps = mmpsum.tile([T, BPC], mybir.dt.float32, tag="ps", name="ps")
                nc.tensor.matmul(ps[:], eh_bf[:], pi[:])
                nxt = pipool.tile([T, BPC], mybir.dt.bfloat16, tag="pi", name="pi")
                nc.vector.tensor_tensor(
                    out=nxt[:], in0=ps[:], in1=fcol(t), op=mybir.AluOpType.mult,
                )
                pi = nxt

                if t % RENORM == 0 and t // RENORM <= NREN and t + APPLY_DELAY < L:
                    ri = t // RENORM - 1
                    # partition sum of pi via ones-matmul on the (mostly idle) PE
                    sp = spsum.tile([1, BPC], mybir.dt.float32, tag="sp", name="sp")
                    nc.tensor.matmul(sp[:], ones_bf[:], pi[:])
                    nc.vector.tensor_copy(out=sacc[:, ri * BPC:(ri + 1) * BPC],
                                          in_=sp[:])
                    rec = rpool.tile([1, BPC], mybir.dt.float32, tag="rec",
                                     name="rec")
                    nc.vector.reciprocal(out=rec[:], in_=sp[:])
                    rb = rpool.tile([T, BPC], mybir.dt.float32, tag="rb", name="rb")
                    nc.gpsimd.partition_broadcast(rb[:], rec[:])
                    # fold 1/S into a future F column, off the critical path
                    ta = t + APPLY_DELAY
                    nc.vector.tensor_tensor(out=fcol(ta), in0=fcol(ta), in1=rb[:],
                                            op=mybir.AluOpType.mult)

            # ---- final: w[b] = sum_i pi[i, b] ----
            spw = spsum.tile([1, BPC], mybir.dt.float32, tag="sp", name="spw")
            nc.tensor.matmul(spw[:], ones_bf[:], pi[:])
            wt = stage.tile([1, BPC], mybir.dt.float32, name="wt")
            nc.vector.tensor_copy(out=wt[:], in_=spw[:])
            nc.sync.dma_start(out=w_out[:, :], in_=wt[:])
            nc.sync.dma_start(out=s_out[:, :], in_=sacc[:])

    nc.compile()
    return nc


_NC_CACHE = None


def _get_nc():
    global _NC_CACHE
    if _NC_CACHE is None:
        _NC_CACHE = _build()
    return _NC_CACHE


def kernel(inputs, tags, mask, transitions, start_transitions, end_transitions):
    logits = np.ascontiguousarray(inputs, dtype=np.float32)
    trans = np.asarray(transitions, dtype=np.float32)
    start_t = np.asarray(start_transitions, dtype=np.float32)
    end_t = np.asarray(end_transitions, dtype=np.float32)
    tags_i = np.asarray(tags).astype(np.int64, copy=False)
    maskf = np.asarray(mask).astype(np.float64)

    # ---------- device part: log-partition via scaled forward ----------
    lg = logits.copy()
    lg[:, 0, :] += start_t[None, :]
    lg[:, -1, :] += end_t[None, :]
    E = np.exp(trans.astype(np.float64))
    ghat = float(np.log(T * E.mean()))
    eh = (E * np.exp(-ghat)).astype(np.float32)
    lg2d = lg.reshape(B * L, T)

    nc = _get_nc()
    in_maps = []
    for c in range(NCORES):
        in_maps.append({
            "lg": lg2d[c * BPC * L:(c + 1) * BPC * L],
            "eh": eh,
        })
    res = run_bass_kernel_spmd(nc, in_maps, core_ids=list(range(NCORES)))

    w = np.stack([res.results[c]["w"] for c in range(NCORES)])     # (8, 1, BPC)
    s = np.stack([res.results[c]["s"] for c in range(NCORES)])     # (8, 1, NREN*BPC)
    logZ = np.log(w.reshape(NCORES * BPC).astype(np.float64))
    srs = s.reshape(NCORES, NREN, BPC).astype(np.float64)
    logZ += np.log(srs).sum(axis=1).reshape(-1)
    logZ += (L - 1) * ghat

    # ---------- host part: gold-path numerator (tiny gathers) ----------
    lf64 = logits.astype(np.float64)
    emit = np.take_along_axis(lf64, tags_i[..., None], axis=2)[..., 0]   # (B, L)
    trans_sc = trans.astype(np.float64)[tags_i[:, :-1], tags_i[:, 1:]]   # (B, L-1)
    score = start_t.astype(np.float64)[tags_i[:, 0]]
    score = score + (trans_sc * maskf[:, 1:]).sum(axis=1)
    score = score + (emit[:, :-1] * maskf[:, :-1]).sum(axis=1)
    last_idx = maskf.astype(np.int64).sum(axis=1) - 1
    last_tags = np.take_along_axis(tags_i, last_idx[:, None], axis=1)[:, 0]
    last_input_score = lf64[np.arange(B), -1, last_tags]
    score = score + end_t.astype(np.float64)[last_tags] + last_input_score * maskf[:, -1]

    return np.float32(np.sum(score - logZ))



# revision 2
# speedup vs baseline: 1.9464x; 1.9464x over previous
"""Trainium2 Bass kernel for nn_ConditionalRandomField_52913997087452.

Computes sum_b [ gold_path_score(b) - log Z(b) ] for a linear-chain CRF with
B=128, L=1024, T=128, mask all-ones.

Strategy (data-parallel over batch, 16 per core x 8 cores), bidirectional:
  - The per-core serial bottleneck is the alpha recurrence's cross-engine
    latency (PE matmul visibility + DVE PSUM-read multiply), ~535 ns/step.
    Instead of one 1023-step forward chain, run TWO independent 511-step
    chains concurrently and meet in the middle:
        forward:  pi_t = f_t * (Ehat^T pi_{t-1}),  t = 1..511
        backward: c_t  = f_t * (Ehat   c_{t+1}),   t = 1022..512
    with f_t = exp(logits_t) (start folded into t=0, end into t=L-1),
    Ehat = exp(transitions - ghat).  Then
        Z * e^{-(L-1) ghat} * (renorm scales) = sum_j (Ehat^T pi_511)[j] * c_512[j].
  - Emissions F are DMA'd in a host-pretransposed [T, B, L] layout (no PE
    transposes on device) and exponentiated in-place-shape by the Act engine.
  - Periodic renormalization: a ones-vector matmul on the PE computes
    S[b] = sum_i v[i,b]; 1/S is folded into a later F column off the critical
    path (Act copy + DVE reciprocal + GPSIMD broadcast/multiply), and S is
    streamed out so the host adds back sum_k log S_k.
  - The gold-path numerator is a tiny gather-and-sum done on the host.

The kernel builder is cached at module level so repeated kernel() calls
reuse the compiled program.
"""
import sys

if "/opt/trn_rl_repo" not in sys.path:
    sys.path.insert(0, "/opt/trn_rl_repo")

import numpy as np

import concourse.bacc as bacc
import concourse.tile as tile
from concourse import mybir
from concourse.bass_utils import run_bass_kernel_spmd

B = 128
L = 1024
T = 128
NCORES = 8
BPC = B // NCORES       # batch per core
NCH = L // 128          # 128-column F chunks
APPLY_DELAY = 8         # fold 1/S into F column +/- APPLY_DELAY steps ahead
NSTEP = 511             # steps per direction
FWD_REN = [128, 256, 384, 480]   # renorm after producing pi_t at these t
BWD_REN = [896, 768, 640, 544]   # renorm after producing c_t at these t
NREN = len(FWD_REN) + len(BWD_REN)
CH_ORDER = [0, 7, 1, 6, 2, 5, 3, 4]  # fwd consumes 0..3, bwd consumes 7..4


def _build():
    nc = bacc.Bacc("TRN2", target_bir_lowering=False)
    # host-pretransposed emissions: [tag, batch, time]
    lg = nc.dram_tensor("lg", [T, BPC, L], mybir.dt.float32, kind="ExternalInput")
    eh = nc.dram_tensor("eh", [T, T], mybir.dt.float32, kind="ExternalInput")
    ehT = nc.dram_tensor("ehT", [T, T], mybir.dt.float32, kind="ExternalInput")
    w_out = nc.dram_tensor("w", [1, BPC], mybir.dt.float32, kind="ExternalOutput")
    s_out = nc.dram_tensor("s", [1, NREN * BPC], mybir.dt.float32,
                           kind="ExternalOutput")

    with tile.TileContext(nc) as tc:
        with (
            tc.tile_pool(name="consts", bufs=1) as consts,
            tc.tile_pool(name="fpool", bufs=1) as fpool,
            tc.tile_pool(name="rawpool", bufs=2) as rawpool,
            tc.tile_pool(name="pipool", bufs=4) as pipool,
            tc.tile_pool(name="cipool", bufs=4) as cipool,
            tc.tile_pool(name="mmpsA", bufs=2, space="PSUM") as mmpsA,
            tc.tile_pool(name="mmpsB", bufs=2, space="PSUM") as mmpsB,
            tc.tile_pool(name="spsum", bufs=2, space="PSUM") as spsum,
            tc.tile_pool(name="rpool", bufs=2) as rpool,
        ):
            # ---- constants ----
            eh_t = consts.tile([T, T], mybir.dt.float32)
            nc.sync.dma_start(out=eh_t[:], in_=eh[:, :])
            ehT_t = consts.tile([T, T], mybir.dt.float32)
            nc.sync.dma_start(out=ehT_t[:], in_=ehT[:, :])
            ones_t = consts.tile([T, 1], mybir.dt.float32)
            nc.vector.memset(ones_t[:], 1.0)
            sacc = consts.tile([1, NREN * BPC], mybir.dt.float32)

            # ---- emissions F: 8 chunk tiles [T, BPC, 128] f32 ----
            F = []
            for c in range(NCH):
                fc = fpool.tile([T, BPC, 128], mybir.dt.float32, tag=f"F{c}",
                                name=f"F{c}")
                F.append(fc)

            def fcol(t):
                return F[t // 128][:, :, t % 128]

            for c in CH_ORDER:
                raw = rawpool.tile([T, BPC, 128], mybir.dt.float32, tag="raw",
                                   name=f"raw{c}")
                nc.sync.dma_start(out=raw[:], in_=lg[:, :, c * 128:(c + 1) * 128])
                qs = range(4) if c < 4 else reversed(range(4))
                for q in qs:  # quarter-granular so chains start sooner
                    nc.scalar.activation(
                        out=F[c][:, :, q * 32:(q + 1) * 32],
                        in_=raw[:, :, q * 32:(q + 1) * 32],
                        func=mybir.ActivationFunctionType.Exp,
                    )

            def renorm(v_ap, slot, fold_t):
                sp = spsum.tile([1, BPC], mybir.dt.float32, tag="sp", name="sp")
                nc.tensor.matmul(sp[:], ones_t[:], v_ap)
                nc.scalar.activation(
                    out=sacc[:, slot * BPC:(slot + 1) * BPC], in_=sp[:],
                    func=mybir.ActivationFunctionType.Copy,
                )
                rec = rpool.tile([1, BPC], mybir.dt.float32, tag="rec",
                                 name="rec")
                nc.vector.reciprocal(out=rec[:], in_=sp[:])
                rb = rpool.tile([T, BPC], mybir.dt.float32, tag="rb", name="rb")
                nc.gpsimd.partition_broadcast(rb[:], rec[:])
                # fold 1/S into a future F column, off the critical path
                nc.gpsimd.tensor_tensor(out=fcol(fold_t), in0=fcol(fold_t),
                                        in1=rb[:], op=mybir.AluOpType.mult)

            # ---- bidirectional recurrence, interleaved emission ----
            pi_ap = fcol(0)       # pi_0 = exp(lg_0 + start)
            ci_ap = fcol(L - 1)   # c_{L-1} = exp(lg_{L-1} + end)
            for k in range(NSTEP):
                tf = k + 1
                tb = L - 2 - k
                psf = mmpsA.tile([T, BPC], mybir.dt.float32, tag="psf",
                                 name="psf")
                nc.tensor.matmul(psf[:], eh_t[:], pi_ap)
                psb = mmpsB.tile([T, BPC], mybir.dt.float32, tag="psb",
                                 name="psb")
                nc.tensor.matmul(psb[:], ehT_t[:], ci_ap)
                npi = pipool.tile([T, BPC], mybir.dt.float32, tag="pi",
                                  name="pi")
                nc.vector.tensor_tensor(out=npi[:], in0=psf[:], in1=fcol(tf),
                                        op=mybir.AluOpType.mult)
                nci = cipool.tile([T, BPC], mybir.dt.float32, tag="ci",
                                  name="ci")
                nc.vector.tensor_tensor(out=nci[:], in0=psb[:], in1=fcol(tb),
                                        op=mybir.AluOpType.mult)
                pi_ap, ci_ap = npi[:], nci[:]

                if tf in FWD_REN:
                    renorm(pi_ap, FWD_REN.index(tf), tf + APPLY_DELAY)
                if tb in BWD_REN:
                    renorm(ci_ap, 4 + BWD_REN.index(tb), tb - APPLY_DELAY)

            # ---- meet in the middle: w[b] = sum_j (Ehat^T pi_511)[j] c_512[j]
            psq = mmpsA.tile([T, BPC], mybir.dt.float32, tag="psf", name="psq")
            nc.tensor.matmul(psq[:], eh_t[:], pi_ap)
            u = pipool.tile([T, BPC], mybir.dt.float32, tag="pi", name="u")
            nc.vector.tensor_tensor(out=u[:], in0=psq[:], in1=ci_ap,
                                    op=mybir.AluOpType.mult)
            spw = spsum.tile([1, BPC], mybir.dt.float32, tag="sp", name="spw")
            nc.tensor.matmul(spw[:], ones_t[:], u[:])
            wt = rpool.tile([1, BPC], mybir.dt.float32, tag="rec", name="wt")
            nc.vector.tensor_copy(out=wt[:], in_=spw[:])
            nc.sync.dma_start(out=w_out[:, :], in_=wt[:])
            nc.sync.dma_start(out=s_out[:, :], in_=sacc[:])

    nc.compile()
    return nc


_NC_CACHE = None


def _get_nc():
    global _NC_CACHE
    if _NC_CACHE is None:
        _NC_CACHE = _build()
    return _NC_CACHE


def kernel(inputs, tags, mask, transitions, start_transitions, end_transitions):
    logits = np.ascontiguousarray(inputs, dtype=np.float32)
    trans = np.asarray(transitions, dtype=np.float32)
    start_t = np.asarray(start_transitions, dtype=np.float32)
    end_t = np.asarray(end_transitions, dtype=np.float32)
    tags_i = np.asarray(tags).astype(np.int64, copy=False)
    maskf = np.asarray(mask).astype(np.float64)

    # ---------- device part: log-partition via bidirectional scaled pass ----
    lg = logits.copy()
    lg[:, 0, :] += start_t[None, :]
    lg[:, -1, :] += end_t[None, :]
    E = np.exp(trans.astype(np.float64))
    ghat = float(np.log(T * E.mean()))
    eh = (E * np.exp(-ghat)).astype(np.float32)
    ehT = np.ascontiguousarray(eh.T)
    # [NCORES, T, BPC, L]: tag-major per core so device DMAs need no transpose
    lgT = np.ascontiguousarray(
        lg.reshape(NCORES, BPC, L, T).transpose(0, 3, 1, 2))

    nc = _get_nc()
    in_maps = []
    for c in range(NCORES):
        in_maps.append({
            "lg": lgT[c],
            "eh": eh,
            "ehT": ehT,
        })
    res = run_bass_kernel_spmd(nc, in_maps, core_ids=list(range(NCORES)))

    w = np.stack([res.results[c]["w"] for c in range(NCORES)])     # (8, 1, BPC)
    s = np.stack([res.results[c]["s"] for c in range(NCORES)])     # (8, 1, NREN*BPC)
    logZ = np.log(w.reshape(NCORES * BPC).astype(np.float64))
    srs = s.reshape(NCORES, NREN, BPC).astype(np.float64)
    logZ += np.log(srs).sum(axis=1).reshape(-1)
    logZ += (L - 1) * ghat

    # ---------- host part: gold-path numerator (tiny gathers) ----------
    lf64 = logits.astype(np.float64)
    emit = np.take_along_axis(lf64, tags_i[..., None], axis=2)[..., 0]   # (B, L)
    trans_sc = trans.astype(np.float64)[tags_i[:, :-1], tags_i[:, 1:]]   # (B, L-1)
    score = start_t.astype(np.float64)[tags_i[:, 0]]
    score = score + (trans_sc * maskf[:, 1:]).sum(axis=1)
    score = score + (emit[:, :-1] * maskf[:, :-1]).sum(axis=1)
    last_idx = maskf.astype(np.int64).sum(axis=1) - 1
    last_tags = np.take_along_axis(tags_i, last_idx[:, None], axis=1)[:, 0]
    last_input_score = lf64[np.arange(B), -1, last_tags]
    score = score + end_t.astype(np.float64)[last_tags] + last_input_score * maskf[:, -1]

    return np.float32(np.sum(score - logZ))


# revision 3
# speedup vs baseline: 1.9977x; 1.0264x over previous
"""Trainium2 Bass kernel for nn_ConditionalRandomField_52913997087452.

Computes sum_b [ gold_path_score(b) - log Z(b) ] for a linear-chain CRF with
B=128, L=1024, T=128, mask all-ones.

Strategy (data-parallel over batch, 16 per core x 8 cores), bidirectional:
  - The per-core serial bottleneck is the alpha recurrence's cross-engine
    latency (PE matmul visibility + DVE PSUM-read multiply), ~535 ns/step.
    Instead of one 1023-step forward chain, run TWO independent chains
    concurrently and meet in the middle:
        forward:  pi_t = f_t * (Ehat^T pi_{t-1}),  t = 1..MID
        backward: c_t  = f_t * (Ehat   c_{t+1}),   t = 1022..MID+1
    with f_t = exp(logits_t) (start folded into t=0, end into t=L-1),
    Ehat = exp(transitions - ghat).  Then per batch column
        Z * e^{-(L-1) ghat} / (renorm scales) = sum_j (Ehat^T pi_MID)[j] * c_{MID+1}[j].
  - Emissions F are DMA'd in a host-pretransposed [T, B, L] layout (no PE
    transposes on device) and exponentiated by the Act engine. The first
    pieces of chunks 0 and 7 are sliced fine so both chains start early.
  - Periodic renormalization: a ones-vector matmul on the PE computes
    S[b] = sum_i v[i,b]; 1/S is folded into a later F column off the critical
    path (Act copy + DVE reciprocal + GPSIMD broadcast/multiply), and S is
    streamed out so the host adds back sum_k log S_k. Fwd/bwd renorm steps
    are staggered so the DVE never sees two renorms at once.
  - The meet product u = (Ehat^T pi_MID) * c_{MID+1} is DMA'd out; the host
    does the final tag-sum and log. The gold-path numerator is a tiny
    gather-and-sum done on the host.

The kernel builder is cached at module level so repeated kernel() calls
reuse the compiled program.
"""
import sys

if "/opt/trn_rl_repo" not in sys.path:
    sys.path.insert(0, "/opt/trn_rl_repo")

import numpy as np

import concourse.bacc as bacc
import concourse.tile as tile
from concourse import mybir
from concourse.bass_utils import run_bass_kernel_spmd

B = 128
L = 1024
T = 128
NCORES = 8
BPC = B // NCORES       # batch per core
NCH = L // 128          # 128-column F chunks
APPLY_DELAY = 8         # fold 1/S into F column +/- APPLY_DELAY steps ahead
MID = 513               # fwd produces pi_1..pi_MID; bwd produces c_1022..c_{MID+1}
NSTEP_F = MID           # 513 fwd multiply steps
NSTEP_B = L - 2 - MID   # 509 bwd multiply steps
FWD_REN = [64, 192, 320, 448]    # renorm after producing pi_t at these t
BWD_REN = [896, 768, 640, 544]   # renorm after producing c_t at these t
NREN = len(FWD_REN) + len(BWD_REN)


def _build():
    nc = bacc.Bacc("TRN2", target_bir_lowering=False)
    # host-pretransposed emissions: [tag, batch, time]
    lg = nc.dram_tensor("lg", [T, BPC, L], mybir.dt.float32, kind="ExternalInput")
    eh = nc.dram_tensor("eh", [T, T], mybir.dt.float32, kind="ExternalInput")
    ehT = nc.dram_tensor("ehT", [T, T], mybir.dt.float32, kind="ExternalInput")
    u_out = nc.dram_tensor("u", [T, BPC], mybir.dt.float32, kind="ExternalOutput")
    s_out = nc.dram_tensor("s", [1, NREN * BPC], mybir.dt.float32,
                           kind="ExternalOutput")

    with tile.TileContext(nc) as tc:
        with (
            tc.tile_pool(name="consts", bufs=1) as consts,
            tc.tile_pool(name="fpool", bufs=1) as fpool,
            tc.tile_pool(name="rawpool", bufs=3) as rawpool,
            tc.tile_pool(name="pipool", bufs=4) as pipool,
            tc.tile_pool(name="cipool", bufs=4) as cipool,
            tc.tile_pool(name="mmpsA", bufs=2, space="PSUM") as mmpsA,
            tc.tile_pool(name="mmpsB", bufs=2, space="PSUM") as mmpsB,
            tc.tile_pool(name="spsum", bufs=2, space="PSUM") as spsum,
            tc.tile_pool(name="rpool", bufs=2) as rpool,
        ):
            # ---- emissions F: 8 chunk tiles [T, BPC, 128] f32 ----
            F = []
            for c in range(NCH):
                fc = fpool.tile([T, BPC, 128], mybir.dt.float32, tag=f"F{c}",
                                name=f"F{c}")
                F.append(fc)

            def fcol(t):
                return F[t // 128][:, :, t % 128]

            raw0 = rawpool.tile([T, BPC, 128], mybir.dt.float32, tag="raw",
                                name="raw0")
            raw7 = rawpool.tile([T, BPC, 128], mybir.dt.float32, tag="raw",
                                name="raw7")

            def dma_piece(rawt, c, lo, hi):
                nc.sync.dma_start(out=rawt[:, :, lo:hi],
                                  in_=lg[:, :, c * 128 + lo:c * 128 + hi])

            def exp_piece(rawt, c, lo, hi):
                nc.scalar.activation(
                    out=F[c][:, :, lo:hi], in_=rawt[:, :, lo:hi],
                    func=mybir.ActivationFunctionType.Exp,
                )

            # fine-sliced first pieces so both chains launch early; the DMA
            # emission order below is the HWDGE grant order
            dma_piece(raw0, 0, 0, 16)
            eh_t = consts.tile([T, T], mybir.dt.float32)
            nc.sync.dma_start(out=eh_t[:], in_=eh[:, :])
            dma_piece(raw7, 7, 112, 128)
            ehT_t = consts.tile([T, T], mybir.dt.float32)
            nc.sync.dma_start(out=ehT_t[:], in_=ehT[:, :])
            exp_piece(raw0, 0, 0, 16)
            exp_piece(raw7, 7, 112, 128)
            dma_piece(raw0, 0, 16, 64)
            dma_piece(raw7, 7, 64, 112)
            exp_piece(raw0, 0, 16, 64)
            exp_piece(raw7, 7, 64, 112)
            dma_piece(raw0, 0, 64, 128)
            dma_piece(raw7, 7, 0, 64)
            exp_piece(raw0, 0, 64, 128)
            exp_piece(raw7, 7, 0, 64)

            ones_t = consts.tile([T, 1], mybir.dt.float32)
            nc.vector.memset(ones_t[:], 1.0)
            sacc = consts.tile([1, NREN * BPC], mybir.dt.float32)

            for c in (1, 6, 2, 5, 3, 4):
                rawc = rawpool.tile([T, BPC, 128], mybir.dt.float32, tag="raw",
                                    name=f"raw{c}")
                nc.sync.dma_start(out=rawc[:],
                                  in_=lg[:, :, c * 128:(c + 1) * 128])
                halves = (0, 1) if c < 4 else (1, 0)
                for h in halves:
                    exp_piece(rawc, c, h * 64, (h + 1) * 64)

            def renorm(v_ap, slot, fold_t):
                sp = spsum.tile([1, BPC], mybir.dt.float32, tag="sp", name="sp")
                nc.tensor.matmul(sp[:], ones_t[:], v_ap)
                nc.scalar.activation(
                    out=sacc[:, slot * BPC:(slot + 1) * BPC], in_=sp[:],
                    func=mybir.ActivationFunctionType.Copy,
                )
                rec = rpool.tile([1, BPC], mybir.dt.float32, tag="rec",
                                 name="rec")
                nc.vector.reciprocal(out=rec[:], in_=sp[:])
                rb = rpool.tile([T, BPC], mybir.dt.float32, tag="rb", name="rb")
                nc.gpsimd.partition_broadcast(rb[:], rec[:])
                # fold 1/S into a future F column, off the critical path
                nc.gpsimd.tensor_tensor(out=fcol(fold_t), in0=fcol(fold_t),
                                        in1=rb[:], op=mybir.AluOpType.mult)

            # ---- bidirectional recurrence, interleaved emission ----
            pi_ap = fcol(0)       # pi_0 = exp(lg_0 + start)
            ci_ap = fcol(L - 1)   # c_{L-1} = exp(lg_{L-1} + end)
            for k in range(NSTEP_F):
                tf = k + 1
                psf = mmpsA.tile([T, BPC], mybir.dt.float32, tag="psf",
                                 name="psf")
                nc.tensor.matmul(psf[:], eh_t[:], pi_ap)
                if k < NSTEP_B:
                    tb = L - 2 - k
                    psb = mmpsB.tile([T, BPC], mybir.dt.float32, tag="psb",
                                     name="psb")
                    nc.tensor.matmul(psb[:], ehT_t[:], ci_ap)
                npi = pipool.tile([T, BPC], mybir.dt.float32, tag="pi",
                                  name="pi")
                nc.vector.tensor_tensor(out=npi[:], in0=psf[:], in1=fcol(tf),
                                        op=mybir.AluOpType.mult)
                pi_ap = npi[:]
                if k < NSTEP_B:
                    nci = cipool.tile([T, BPC], mybir.dt.float32, tag="ci",
                                      name="ci")
                    nc.vector.tensor_tensor(out=nci[:], in0=psb[:],
                                            in1=fcol(tb),
                                            op=mybir.AluOpType.mult)
                    ci_ap = nci[:]

                if tf in FWD_REN:
                    renorm(pi_ap, FWD_REN.index(tf), tf + APPLY_DELAY)
                if k < NSTEP_B and tb in BWD_REN:
                    renorm(ci_ap, 4 + BWD_REN.index(tb), tb - APPLY_DELAY)

            # ---- meet in the middle: u = (Ehat^T pi_MID) * c_{MID+1} ----
            psq = mmpsA.tile([T, BPC], mybir.dt.float32, tag="psf", name="psq")
            nc.tensor.matmul(psq[:], eh_t[:], pi_ap)
            u = pipool.tile([T, BPC], mybir.dt.float32, tag="pi", name="u")
            nc.vector.tensor_tensor(out=u[:], in0=psq[:], in1=ci_ap,
                                    op=mybir.AluOpType.mult)
            nc.sync.dma_start(out=s_out[:, :], in_=sacc[:])
            nc.sync.dma_start(out=u_out[:, :], in_=u[:])

    nc.compile()
    return nc


_NC_CACHE = None


def _get_nc():
    global _NC_CACHE
    if _NC_CACHE is None:
        _NC_CACHE = _build()
    return _NC_CACHE


def kernel(inputs, tags, mask, transitions, start_transitions, end_transitions):
    logits = np.ascontiguousarray(inputs, dtype=np.float32)
    trans = np.asarray(transitions, dtype=np.float32)
    start_t = np.asarray(start_transitions, dtype=np.float32)
    end_t = np.asarray(end_transitions, dtype=np.float32)
    tags_i = np.asarray(tags).astype(np.int64, copy=False)
    maskf = np.asarray(mask).astype(np.float64)

    # ---------- device part: log-partition via bidirectional scaled pass ----
    lg = logits.copy()
    lg[:, 0, :] += start_t[None, :]
    lg[:, -1, :] += end_t[None, :]
    E = np.exp(trans.astype(np.float64))
    ghat = float(np.log(T * E.mean()))
    eh = (E * np.exp(-ghat)).astype(np.float32)
    ehT = np.ascontiguousarray(eh.T)
    # [NCORES, T, BPC, L]: tag-major per core so device DMAs need no transpose
    lgT = np.ascontiguousarray(
        lg.reshape(NCORES, BPC, L, T).transpose(0, 3, 1, 2))

    nc = _get_nc()
    in_maps = []
    for c in range(NCORES):
        in_maps.append({
            "lg": lgT[c],
            "eh": eh,
            "ehT": ehT,
        })
    res = run_bass_kernel_spmd(nc, in_maps, core_ids=list(range(NCORES)))

    u = np.stack([res.results[c]["u"] for c in range(NCORES)])     # (8, T, BPC)
    s = np.stack([res.results[c]["s"] for c in range(NCORES)])     # (8, 1, NREN*BPC)
    w = u.astype(np.float64).sum(axis=1)                           # (8, BPC)
    logZ = np.log(w.reshape(NCORES * BPC))
    srs = s.reshape(NCORES, NREN, BPC).astype(np.float64)
    logZ += np.log(srs).sum(axis=1).reshape(-1)
    logZ += (L - 1) * ghat

    # ---------- host part: gold-path numerator (tiny gathers) ----------
    lf64 = logits.astype(np.float64)
    emit = np.take_along_axis(lf64, tags_i[..., None], axis=2)[..., 0]   # (B, L)
    trans_sc = trans.astype(np.float64)[tags_i[:, :-1], tags_i[:, 1:]]   # (B, L-1)
    score = start_t.astype(np.float64)[tags_i[:, 0]]
    score = score + (trans_sc * maskf[:, 1:]).sum(axis=1)
    score = score + (emit[:, :-1] * maskf[:, :-1]).sum(axis=1)
    last_idx = maskf.astype(np.int64).sum(axis=1) - 1
    last_tags = np.take_along_axis(tags_i, last_idx[:, None], axis=1)[:, 0]
    last_input_score = lf64[np.arange(B), -1, last_tags]
    score = score + end_t.astype(np.float64)[last_tags] + last_input_score * maskf[:, -1]

    return np.float32(np.sum(score - logZ))
